# revision 39
# baseline (speedup 1.0000x reference)
"""DiffAttn2d TRN2 Bass kernel (v2).

Sharding: 8 cores = 2 (batch) x 4 (head-groups of 2 heads / 4 doubled-heads).

Per core (channel-major scores [key j, query i], n=2048, 4 doubled heads):
  - dots^T via row-packed K=16 fp32r matmuls (4 dheads in 4 PE row groups)
  - one ACT exp pass per (jc, dh) dots tile PSUM->SBUF bf16 (the only O(n^2)
    elementwise op; ACT is the bottleneck engine at ~133us busy)
  - attn @ v with the EXP TILE STATIONARY: out z^T[i, c] = sum_j exp[j,i] v[j,c]
    -> 32-row matmuls (cost model charges moving/out rows only).  32 slot
    accumulators interleave in 2 PSUM banks; a single bank-starting matmul
    (jc==0, slot 0/16) lazily zeroes the bank, remaining slots' first writes
    init via the pending-zero flag.  Softmax denominators from 1-row
    ones-matmuls accumulated in [128, 32] PSUM the same way.
  - epilogue in [i-part, c-free] domain on DVE (two pipelined halves):
    reciprocal, lambda-fold, stride-0 broadcast mults, strided even-odd
    subtract, free-dim reduces for LN stats; rsqrt via tiny [128, 8] Ln/Exp
    on ACT (zero table switches)
  - PE f32 transposes (identity moving) bring u' back to [c, i] for
    gamma/beta, sigmoid gating, and the K=32 output projection
  - gates: sigmoid = 1/(1+e^-g): ACT only does e^-g; +1 on Pool; reciprocal
    on DVE for ip0 and ones-divide on Pool for ip1 (tail, off both
    bottleneck engines)
  - y^T partials summed on host (+ bout)
"""
import sys
sys.path.insert(0, "/opt/trn_rl_repo")

import math
import numpy as np
import ml_dtypes

import concourse.bass as bass
import concourse.bacc as bacc_mod
import concourse.mybir as mybir
from concourse.tile import TileContext
from concourse.bass_utils import run_bass_kernel_spmd

F = mybir.dt.float32
R = mybir.dt.float32r
BF = mybir.dt.bfloat16
AF = mybir.ActivationFunctionType
AL = mybir.AluOpType
AX = mybir.AxisListType

H, DH = 8, 16
DEPTH = 1
LAMBDA_INIT = 0.8 - 0.6 * math.exp(-0.3 * DEPTH)
LN_EPS = 1e-5
B, N, DIM = 2, 2048, 256
NC = 8

_cached = {}


def _bcast(ap, n):
    """Append a stride-0 free dim of size n (broadcast along free)."""
    return bass.AP(ap.tensor, ap.offset, list(ap.ap) + [[0, n]])


def build_kernel(beta_zero=True):
    nc = bacc_mod.Bacc()
    xT = nc.declare_dram_parameter("xT", [DIM, N], R, isOutput=False)
    wqp = nc.declare_dram_parameter("wqp", [DIM, 128], R, isOutput=False)
    wkp = nc.declare_dram_parameter("wkp", [DIM, 128], R, isOutput=False)
    wv = nc.declare_dram_parameter("wv", [DIM, 64], R, isOutput=False)
    wgc = nc.declare_dram_parameter("wgc", [DIM, 64], R, isOutput=False)
    wout = nc.declare_dram_parameter("wout", [32, 2, 2, 128], R, isOutput=False)
    ident = nc.declare_dram_parameter("ident", [128, 128], F, isOutput=False)
    lmask = nc.declare_dram_parameter("lmask", [128, 32], F, isOutput=False)
    gam = nc.declare_dram_parameter("gam", [32, 1], F, isOutput=False)
    bet = nc.declare_dram_parameter("bet", [32, 1], F, isOutput=False)
    nbg = nc.declare_dram_parameter("nbg", [32, 2], F, isOutput=False)
    pbg = nc.declare_dram_parameter("pbg", [32, 2], F, isOutput=False)
    epsc = nc.declare_dram_parameter("epsc", [128, 1], F, isOutput=False)
    yT = nc.declare_dram_parameter("yT", [DIM, N], F, isOutput=True)

    with TileContext(nc) as tc:
        with tc.tile_pool(name="pers", bufs=1) as pers, \
             tc.tile_pool(name="ebp", bufs=12) as ebp, \
             tc.tile_pool(name="ep", bufs=2) as ep, \
             tc.tile_pool(name="dp", bufs=2, space="PSUM") as dp, \
             tc.tile_pool(name="zp", bufs=1, space="PSUM") as zp, \
             tc.tile_pool(name="tp", bufs=1, space="PSUM") as tpp, \
             tc.tile_pool(name="sp", bufs=1, space="PSUM") as spp:

            # prewarm ACT exp/ln table set
            warm = pers.tile([1, 8], F, tag="warm")
            nc.vector.memset(warm[:], 0.0)
            nc.scalar.activation(warm[:], warm[:], AF.Exp)

            # ---------------- DMAs (ordered for earliest projections) -------
            xt = pers.tile([128, 2, N], R, tag="xt")
            xTr = xT.rearrange("(f p) n -> p f n", p=128)
            twkp = pers.tile([128, 2, 128], R, tag="twkp")
            nc.sync.dma_start(out=twkp[:], in_=wkp.rearrange("(f p) m -> p f m", p=128))
            twqp = pers.tile([128, 2, 128], R, tag="twqp")
            nc.sync.dma_start(out=twqp[:], in_=wqp.rearrange("(f p) m -> p f m", p=128))
            for c in range(2):
                for f in range(2):
                    nc.sync.dma_start(out=xt[:, f, c * 512:(c + 1) * 512],
                                      in_=xTr[:, f, c * 512:(c + 1) * 512])
            twv = pers.tile([128, 2, 64], R, tag="twv")
            nc.sync.dma_start(out=twv[:], in_=wv.rearrange("(f p) m -> p f m", p=128))
            twg = pers.tile([128, 2, 64], R, tag="twg")
            nc.sync.dma_start(out=twg[:], in_=wgc.rearrange("(f p) m -> p f m", p=128))
            for c in range(2, 4):
                for f in range(2):
                    nc.sync.dma_start(out=xt[:, f, c * 512:(c + 1) * 512],
                                      in_=xTr[:, f, c * 512:(c + 1) * 512])
            tout = pers.tile([32, 2, 2, 128], R, tag="tout")
            nc.sync.dma_start(out=tout[:], in_=wout[:])
            tid = pers.tile([128, 128], F, tag="tid")
            nc.sync.dma_start(out=tid[:], in_=ident[:])
            tlm = pers.tile([128, 32], F, tag="tlm")
            nc.sync.dma_start(out=tlm[:], in_=lmask[:])
            tgam = pers.tile([32, 1], F, tag="tgam")
            nc.sync.dma_start(out=tgam[:], in_=gam[:])
            tbet = pers.tile([32, 1], F, tag="tbet")
            nc.sync.dma_start(out=tbet[:], in_=bet[:])
            tnbg = pers.tile([32, 2], F, tag="tnbg")
            nc.sync.dma_start(out=tnbg[:], in_=nbg[:])
            tpbg = pers.tile([32, 2], F, tag="tpbg")
            nc.sync.dma_start(out=tpbg[:], in_=pbg[:])
            teps = pers.tile([128, 1], F, tag="teps")
            nc.sync.dma_start(out=teps[:], in_=epsc[:])
            tones = pers.tile([128, 1], BF, tag="tones")
            nc.vector.memset(tones[:], 1.0)

            # ---------------- projections ----------------
            # q^T / k^T packed: partition 32d+j (j<16) = channel j of dhead d
            qTp = pers.tile([128, N], R, tag="qTp")
            kTp = pers.tile([128, N], R, tag="kTp")
            vpp = pers.tile([128, 16, 64], BF, tag="vpp")
            sge = pers.tile([32, 4, 1024], F, tag="sge")
            sgq = pers.tile([32, 4, 1024], F, tag="sgq")

            def proj_qk(dst, w, it):
                ps = dp.tile([128, 1024], F, tag="dots")
                for f in range(2):
                    nc.tensor.matmul(ps[:, 0:512], w[:, f, :],
                                     xt[:, f, it * 512:(it + 1) * 512],
                                     start=(f == 0), stop=(f == 1))
                nc.vector.tensor_copy(dst[:, it * 512:(it + 1) * 512], ps[:, 0:512])

            def proj_v(jc):
                ps = dp.tile([128, 1024], F, tag="dots")
                for f in range(2):
                    nc.tensor.matmul(ps[:, 0:64], xt[:, f, jc * 128:(jc + 1) * 128],
                                     twv[:, f, :], start=(f == 0), stop=(f == 1))
                nc.vector.tensor_copy(vpp[:, jc, :], ps[:, 0:64])

            def gates_mm(ip, b):
                ps = dp.tile([128, 1024], F, tag="dots")
                for nt in range(2):
                    for f in range(2):
                        nc.tensor.matmul(
                            ps[0:32, nt * 512:(nt + 1) * 512],
                            twg[:, f, 32 * b:32 * b + 32],
                            xt[:, f, ip * 1024 + nt * 512:ip * 1024 + (nt + 1) * 512],
                            start=(f == 0), stop=(f == 1))
                nc.scalar.activation(sge[:, 2 * ip + b, :], ps[0:32, :], AF.Exp,
                                     scale=-1.0, bias=tnbg[:, b:b + 1])

            proj_qk(kTp, twkp, 0)
            proj_qk(qTp, twqp, 0)
            proj_qk(qTp, twqp, 1)
            proj_v(0)

            def proj_rest():
                items = [lambda: proj_v(1),
                         lambda: proj_v(2), lambda: proj_v(3),
                         lambda: proj_qk(kTp, twkp, 1), lambda: proj_v(4), lambda: proj_v(5),
                         lambda: proj_qk(kTp, twkp, 2), lambda: proj_v(6), lambda: proj_v(7),
                         lambda: proj_qk(kTp, twkp, 3), lambda: proj_v(8), lambda: proj_v(9),
                         lambda: proj_qk(qTp, twqp, 2), lambda: proj_v(10), lambda: proj_v(11),
                         lambda: proj_qk(qTp, twqp, 3), lambda: proj_v(12), lambda: proj_v(13),
                         lambda: proj_v(14), lambda: proj_v(15),
                         lambda: gates_mm(0, 0), lambda: gates_mm(0, 1)]
                for i, it in enumerate(items):
                    it()
                    if i >= 10 or i % 2 == 1:
                        yield

            # ---------------- attention ----------------
            saved = {}

            def attention(ip, interleave=None):
                i0 = ip * 1024
                zreg = zp.tile([128, 32, 32], F, tag="zreg")
                sreg = spp.tile([128, 32], F, tag="sreg")

                def zs_mm(jc, ebs):
                    # z/s matmuls lag the dots of the NEXT jc so the PE never
                    # blocks on the current jc's last exp
                    for dh in range(4):
                        pair = dh // 2
                        eb = ebs[dh]
                        for ib in range(8):
                            slot = ib * 4 + dh
                            st = eb[:, ib * 128:(ib + 1) * 128]
                            nc.tensor.matmul(
                                zreg[:, slot, :], st,
                                vpp[:, jc, 32 * pair:32 * pair + 32],
                                start=(jc == 0 and slot in (0, 16)),
                                stop=(jc == 15),
                                skip_group_check=True,
                            )
                            nc.tensor.matmul(
                                sreg[:, slot:slot + 1], st, tones[:],
                                start=(jc == 0 and slot == 0),
                                stop=(jc == 15),
                                skip_group_check=True,
                            )

                prev = None
                for jc in range(16):
                    j0 = jc * 128
                    ebs = []
                    if ip == 0 and jc == 0:
                        # first tile h-split: ACT starts on the q0 halves
                        # before the q1 projection lands.  Two psum tiles
                        # pack (dh0,dh1) / (dh2,dh3) per h-half.
                        ebs = [ebp.tile([128, 1024], BF, tag="ebf", name=f"eb0_{k}")
                               for k in range(4)]
                        for h in range(2):
                            dts = [dp.tile([128, 1024], F, tag="dots", name=f"dt0_{h}{k}")
                                   for k in range(2)]
                            for dh in range(4):
                                r0 = 32 * dh
                                nc.tensor.matmul(
                                    dts[dh // 2][:, (dh % 2) * 512:(dh % 2 + 1) * 512],
                                    kTp[r0:r0 + 16, j0:j0 + 128],
                                    qTp[r0:r0 + 16, i0 + h * 512:i0 + (h + 1) * 512],
                                    start=True, stop=True,
                                    tile_position=(r0, 0),
                                )
                            for dh in range(4):
                                nc.scalar.activation(
                                    ebs[dh][:, h * 512:(h + 1) * 512],
                                    dts[dh // 2][:, (dh % 2) * 512:(dh % 2 + 1) * 512],
                                    AF.Exp)
                    else:
                        for dh in range(4):
                            r0 = 32 * dh
                            dt_ = dp.tile([128, 1024], F, tag="dots")
                            for h in range(2):
                                nc.tensor.matmul(
                                    dt_[:, h * 512:(h + 1) * 512],
                                    kTp[r0:r0 + 16, j0:j0 + 128],
                                    qTp[r0:r0 + 16, i0 + h * 512:i0 + (h + 1) * 512],
                                    start=True, stop=True,
                                    tile_position=(r0, 0),
                                )
                            eb = ebp.tile([128, 1024], BF, tag="ebf")
                            nc.scalar.activation(eb[:], dt_[:], AF.Exp)
                            ebs.append(eb)
                    if prev is not None:
                        zs_mm(jc - 1, prev)
                    prev = ebs
                    if interleave is not None:
                        next(interleave, None)
                zs_mm(15, prev)
                saved[ip] = (zreg, sreg)

            def epi_dve(ip):
                """u/LN chain in [i, c] domain, split in two pipelined halves
                (pairslots 0:8 | 8:16 = slots 0:16 | 16:32)."""
                zreg, sreg = saved[ip]
                ucs = []
                for hb in range(2):
                    s0, ps0 = 16 * hb, 8 * hb
                    sfx = f"{ip}{hb}"
                    rinv = ep.tile([128, 16], F, tag=f"rinv{hb}", name=f"rinv{sfx}")
                    nc.vector.reciprocal(rinv[:], sreg[:, s0:s0 + 16])
                    rinl = ep.tile([128, 16], F, tag=f"rinl{hb}", name=f"rinl{sfx}")
                    nc.vector.tensor_tensor(rinl[:], rinv[:], tlm[:, s0:s0 + 16], AL.mult)
                    zr = ep.tile([128, 16, 32], F, tag=f"zr{hb}", name=f"zr{sfx}")
                    nc.vector.tensor_tensor(zr[:], zreg[:, s0:s0 + 16, :],
                                            _bcast(rinl[:], 32), AL.mult)
                    u = ep.tile([128, 8, 32], F, tag=f"u{hb}", name=f"u{sfx}")
                    nc.vector.tensor_tensor(u[:], zr[:, 0:16:2, :], zr[:, 1:16:2, :],
                                            AL.subtract)
                    usq = ep.tile([128, 8, 32], F, tag=f"usq{hb}", name=f"usq{sfx}")
                    nc.vector.tensor_tensor(usq[:], u[:], u[:], AL.mult)
                    s1 = ep.tile([128, 8], F, tag=f"s1{hb}", name=f"s1{sfx}")
                    nc.vector.tensor_reduce(s1[:], u[:], AX.X, AL.add)
                    s2 = ep.tile([128, 8], F, tag=f"s2{hb}", name=f"s2{sfx}")
                    nc.vector.tensor_reduce(s2[:], usq[:], AX.X, AL.add)
                    mu = ep.tile([128, 8], F, tag=f"mu{hb}", name=f"mu{sfx}")
                    nc.vector.tensor_scalar(mu[:], s1[:], 1.0 / 32.0, None, AL.mult)
                    var = ep.tile([128, 8], F, tag=f"var{hb}", name=f"var{sfx}")
                    nc.vector.tensor_tensor(var[:], mu[:], mu[:], AL.mult)
                    nc.vector.tensor_scalar(var[:], var[:], 32.0, None, AL.mult)
                    nc.vector.tensor_tensor(var[:], s2[:], var[:], AL.subtract)
                    w = ep.tile([128, 8], F, tag=f"w{hb}", name=f"w{sfx}")
                    nc.scalar.activation(var[:], var[:], AF.Ln, scale=1.0 / 32.0,
                                         bias=teps[:])
                    nc.scalar.activation(w[:], var[:], AF.Exp, scale=-0.5)
                    uc = ep.tile([128, 8, 32], F, tag=f"uc{hb}", name=f"uc{sfx}")
                    nc.vector.tensor_tensor(uc[:], u[:], _bcast(mu[:], 32), AL.subtract)
                    nc.vector.tensor_tensor(uc[:], uc[:], _bcast(w[:], 32), AL.mult)
                    ucs.append(uc)
                saved[ip] = ucs

            def epi_out(ip):
                """u'^T via PE transposes, gamma/beta + gating on DVE, output
                projection per 512-col chunk; yields between chunks."""
                ucs = saved[ip]
                gr2 = ep.tile([32, 2, 1024], R, tag="gr2", name=f"gr2_{ip}")
                # one psum bank holding two manual chunk buffers (region-level
                # dep tracking pipelines transposes against the gating mults)
                tpt = tpp.tile([32, 2, 2, 128], F, tag="tpt", name=f"tpt_{ip}")
                for nt in range(2):
                    for ib4 in range(4):
                        ib = nt * 4 + ib4
                        bb = ib % 2
                        for pair in range(2):
                            psl = (ib * 2 + pair) % 8
                            nc.tensor.transpose(tpt[:, bb, pair, :],
                                                ucs[ib // 4][:, psl, :], tid[:])
                        c0 = ib * 128
                        if beta_zero:
                            # gamma folded into wout on the host; beta==0:
                            # gating is a single mult straight off the psum
                            nc.vector.tensor_tensor(
                                gr2[:, :, c0:c0 + 128], tpt[:, bb, :, :],
                                sgq[:, 2 * ip:2 * ip + 2, c0:c0 + 128], AL.mult)
                        else:
                            gg = ep.tile([32, 2, 128], F, tag="gg",
                                         name=f"gg_{ip}{ib}")
                            nc.vector.tensor_scalar(gg[:], tpt[:, bb, :, :], tgam[:],
                                                    tbet[:], AL.mult, AL.add)
                            nc.vector.tensor_tensor(
                                gr2[:, :, c0:c0 + 128], gg[:],
                                sgq[:, 2 * ip:2 * ip + 2, c0:c0 + 128], AL.mult)
                        if ib % 2 == 1:
                            yield
                    yp = dp.tile([128, 1024], F, tag="dots")
                    for oh in range(2):
                        for pair in range(2):
                            nc.tensor.matmul(yp[:, oh * 512:(oh + 1) * 512],
                                             tout[:, pair, oh, :],
                                             gr2[:, pair, nt * 512:(nt + 1) * 512],
                                             start=(pair == 0), stop=(pair == 1))
                    ys = ep.tile([128, 1024], F, tag=f"ys{nt}", name=f"ys{nt}_{ip}")
                    if ip == 1 and nt == 0:
                        # keep DVE free for the ongoing epilogue cascade:
                        # copies via idle ACT
                        nc.scalar.copy(ys[:, 0:512], yp[:, 0:512])
                        nc.scalar.copy(ys[:, 512:1024], yp[:, 512:1024])
                    elif ip == 1:
                        # last chunk: split across ACT and DVE
                        nc.scalar.copy(ys[:, 0:512], yp[:, 0:512])
                        nc.vector.tensor_copy(ys[:, 512:1024], yp[:, 512:1024])
                    else:
                        nc.vector.tensor_copy(ys[:], yp[:])
                    c0 = ip * 1024 + nt * 512
                    nc.sync.dma_start(out=yT[0:128, c0:c0 + 512], in_=ys[:, 0:512])
                    eng = nc.scalar if ip == 1 else nc.sync
                    eng.dma_start(out=yT[128:256, c0:c0 + 512], in_=ys[:, 512:1024])
                    yield

            gen_proj = proj_rest()
            attention(0, interleave=gen_proj)
            for _ in gen_proj:
                pass
            # finish sigmoid for ip0 off the critical engines
            nc.gpsimd.tensor_scalar(sge[:, 0:2, :], sge[:, 0:2, :], 1.0, None, AL.add)
            nc.vector.reciprocal(sgq[:, 0:2, :], sge[:, 0:2, :])
            epi_dve(0)
            gen0 = epi_out(0)

            class InterleaveAt:
                """Step gen0 only during the later jc's of attention(1)."""
                def __init__(self, gen, start):
                    self.gen, self.start, self.jc = gen, start, 0
                def __next__(self):
                    if self.jc >= self.start:
                        next(self.gen, None)
                    self.jc += 1
                    return None

            attention(1, interleave=InterleaveAt(gen0, 8))
            for _ in gen0:
                pass
            # gates for ip1: the whole sigmoid runs on ACT inside the
            # post-attention ACT gap: sig = exp(-ln(exp(-(g+bg)) + 1))
            gates_mm(1, 0)
            gates_mm(1, 1)
            nc.scalar.activation(sge[:, 2:4, :], sge[:, 2:4, :], AF.Ln, bias=1.0)
            nc.scalar.activation(sgq[:, 2:4, :], sge[:, 2:4, :], AF.Exp, scale=-1.0)
            epi_dve(1)
            for _ in epi_out(1):
                pass

    nc.finalize()
    return nc


def _prep_core_inputs(inputs, bi, hg, lam, beta_zero=True):
    scale = DH ** -0.5
    x = np.asarray(inputs["x"], np.float32)
    Wq = np.asarray(inputs["Wq"], np.float32)
    Wkv = np.asarray(inputs["Wkv"], np.float32)
    Wout = np.asarray(inputs["Wout"], np.float32)
    Wg = np.asarray(inputs["Wg"], np.float32)
    bg = np.asarray(inputs["bg"], np.float32)
    g_ = np.asarray(inputs["ln_gamma"], np.float32)
    b_ = np.asarray(inputs["ln_beta"], np.float32)
    li = np.float32(1.0 - LAMBDA_INIT)

    c0 = 64 * hg
    wq_c = Wq[:, c0:c0 + 64] * scale
    wk_c = Wkv[:, c0:c0 + 64]
    wv_c = Wkv[:, 256 + c0:256 + c0 + 64]
    wg_c = Wg[:, c0:c0 + 64]
    wout_c = Wout[c0:c0 + 64, :]

    wqp = np.zeros((256, 128), np.float32)
    wkp = np.zeros((256, 128), np.float32)
    for d in range(4):
        wqp[:, 32 * d:32 * d + 16] = wq_c[:, 16 * d:16 * d + 16]
        wkp[:, 32 * d:32 * d + 16] = wk_c[:, 16 * d:16 * d + 16]

    gvec = (g_[0:32] * li).astype(np.float32)
    woutp = np.zeros((32, 2, 2, 128), np.float32)
    for pair in range(2):
        for oh in range(2):
            woutp[:, pair, oh, :] = wout_c[32 * pair:32 * pair + 32,
                                           128 * oh:128 * oh + 128]
            if beta_zero:
                # gamma (and the 1-lambda_init factor) folded into wout
                woutp[:, pair, oh, :] *= gvec[:, None]

    lmask = np.ones((128, 32), np.float32)
    for slot in range(32):
        if slot % 4 in (1, 3):
            lmask[:, slot] = lam

    nbg = np.zeros((32, 2), np.float32)
    pbg = np.zeros((32, 2), np.float32)
    for b in range(2):
        nbg[:, b] = -bg[c0 + 32 * b:c0 + 32 * b + 32]
        pbg[:, b] = bg[c0 + 32 * b:c0 + 32 * b + 32]

    return {
        "xT": np.ascontiguousarray(x[bi].T),
        "wqp": wqp, "wkp": wkp,
        "wv": np.ascontiguousarray(wv_c),
        "wgc": np.ascontiguousarray(wg_c),
        "wout": woutp,
        "ident": np.eye(128, dtype=np.float32),
        "lmask": lmask,
        "gam": (g_[0:32] * li).reshape(32, 1).astype(np.float32),
        "bet": (b_[0:32] * li).reshape(32, 1).astype(np.float32),
        "nbg": nbg, "pbg": pbg,
        "epsc": np.full((128, 1), LN_EPS, np.float32),
    }


def kernel(**inputs) -> np.ndarray:
    lq1 = np.asarray(inputs["lq1"], np.float64)
    lk1 = np.asarray(inputs["lk1"], np.float64)
    lq2 = np.asarray(inputs["lq2"], np.float64)
    lk2 = np.asarray(inputs["lk2"], np.float64)
    lam = float(np.exp(np.sum(lq1 * lk1)) - np.exp(np.sum(lq2 * lk2)) + LAMBDA_INIT)
    bout = np.asarray(inputs["bout"], np.float32)
    beta_zero = bool(np.all(np.asarray(inputs["ln_beta"]) == 0.0))

    key = ("nc", beta_zero)
    if key not in _cached:
        _cached[key] = build_kernel(beta_zero)
    nc = _cached[key]

    in_maps = []
    for c in range(NC):
        bi, hg = c // 4, c % 4
        in_maps.append(_prep_core_inputs(inputs, bi, hg, lam, beta_zero))

    import os
    trace = bool(int(os.environ.get("BASS_KERNEL_TRACE", "0")))
    res = run_bass_kernel_spmd(nc, in_maps, list(range(NC)), trace=trace)
    _cached["exec_time_ns"] = res.exec_time_ns
    _cached["trace"] = res.instructions_and_trace
    out = np.zeros((B, N, DIM), np.float32)
    for c in range(NC):
        bi = c // 4
        out[bi] += res.results[c]["yT"].T
    out += bout
    return out


# revision 40
# speedup vs baseline: 1.0049x; 1.0049x over previous
"""DiffAttn2d TRN2 Bass kernel (v2).

Sharding: 8 cores = 2 (batch) x 4 (head-groups of 2 heads / 4 doubled-heads).

Per core (channel-major scores [key j, query i], n=2048, 4 doubled heads):
  - dots^T via row-packed K=16 fp32r matmuls (4 dheads in 4 PE row groups)
  - one ACT exp pass per (jc, dh) dots tile PSUM->SBUF bf16 (the only O(n^2)
    elementwise op; ACT is the bottleneck engine at ~133us busy)
  - attn @ v with the EXP TILE STATIONARY: out z^T[i, c] = sum_j exp[j,i] v[j,c]
    -> 32-row matmuls (cost model charges moving/out rows only).  32 slot
    accumulators interleave in 2 PSUM banks; a single bank-starting matmul
    (jc==0, slot 0/16) lazily zeroes the bank, remaining slots' first writes
    init via the pending-zero flag.  Softmax denominators from 1-row
    ones-matmuls accumulated in [128, 32] PSUM the same way.
  - epilogue in [i-part, c-free] domain on DVE (two pipelined halves):
    reciprocal, lambda-fold, stride-0 broadcast mults, strided even-odd
    subtract, free-dim reduces for LN stats; rsqrt via tiny [128, 8] Ln/Exp
    on ACT (zero table switches)
  - PE f32 transposes (identity moving) bring u' back to [c, i] for
    gamma/beta, sigmoid gating, and the K=32 output projection
  - gates: sigmoid = 1/(1+e^-g): ACT only does e^-g; +1 on Pool; reciprocal
    on DVE for ip0 and ones-divide on Pool for ip1 (tail, off both
    bottleneck engines)
  - y^T partials summed on host (+ bout)
"""
import sys
sys.path.insert(0, "/opt/trn_rl_repo")

import math
import numpy as np
import ml_dtypes

import concourse.bass as bass
import concourse.bacc as bacc_mod
import concourse.mybir as mybir
from concourse.tile import TileContext
from concourse.bass_utils import run_bass_kernel_spmd

F = mybir.dt.float32
R = mybir.dt.float32r
BF = mybir.dt.bfloat16
AF = mybir.ActivationFunctionType
AL = mybir.AluOpType
AX = mybir.AxisListType

H, DH = 8, 16
DEPTH = 1
LAMBDA_INIT = 0.8 - 0.6 * math.exp(-0.3 * DEPTH)
LN_EPS = 1e-5
B, N, DIM = 2, 2048, 256
NC = 8

_cached = {}


def _bcast(ap, n):
    """Append a stride-0 free dim of size n (broadcast along free)."""
    return bass.AP(ap.tensor, ap.offset, list(ap.ap) + [[0, n]])


def build_kernel(beta_zero=True):
    nc = bacc_mod.Bacc()
    xT = nc.declare_dram_parameter("xT", [DIM, N], R, isOutput=False)
    wqp = nc.declare_dram_parameter("wqp", [DIM, 128], R, isOutput=False)
    wkp = nc.declare_dram_parameter("wkp", [DIM, 128], R, isOutput=False)
    wv = nc.declare_dram_parameter("wv", [DIM, 64], R, isOutput=False)
    wgc = nc.declare_dram_parameter("wgc", [DIM, 64], R, isOutput=False)
    wout = nc.declare_dram_parameter("wout", [32, 2, 2, 128], R, isOutput=False)
    ident = nc.declare_dram_parameter("ident", [128, 128], F, isOutput=False)
    lmask = nc.declare_dram_parameter("lmask", [128, 32], F, isOutput=False)
    gam = nc.declare_dram_parameter("gam", [32, 1], F, isOutput=False)
    bet = nc.declare_dram_parameter("bet", [32, 1], F, isOutput=False)
    nbg = nc.declare_dram_parameter("nbg", [32, 2], F, isOutput=False)
    pbg = nc.declare_dram_parameter("pbg", [32, 2], F, isOutput=False)
    epsc = nc.declare_dram_parameter("epsc", [128, 1], F, isOutput=False)
    yT = nc.declare_dram_parameter("yT", [DIM, N], F, isOutput=True)

    with TileContext(nc) as tc:
        with tc.tile_pool(name="pers", bufs=1) as pers, \
             tc.tile_pool(name="ebp", bufs=12) as ebp, \
             tc.tile_pool(name="ep", bufs=2) as ep, \
             tc.tile_pool(name="dp", bufs=2, space="PSUM") as dp, \
             tc.tile_pool(name="zp", bufs=1, space="PSUM") as zp, \
             tc.tile_pool(name="tp", bufs=1, space="PSUM") as tpp, \
             tc.tile_pool(name="sp", bufs=1, space="PSUM") as spp:

            # prewarm ACT exp/ln table set
            warm = pers.tile([1, 8], F, tag="warm")
            nc.vector.memset(warm[:], 0.0)
            nc.scalar.activation(warm[:], warm[:], AF.Exp)

            # ---------------- DMAs (ordered for earliest projections) -------
            xt = pers.tile([128, 2, N], R, tag="xt")
            xTr = xT.rearrange("(f p) n -> p f n", p=128)
            twkp = pers.tile([128, 2, 128], R, tag="twkp")
            nc.sync.dma_start(out=twkp[:], in_=wkp.rearrange("(f p) m -> p f m", p=128))
            twqp = pers.tile([128, 2, 128], R, tag="twqp")
            nc.sync.dma_start(out=twqp[:], in_=wqp.rearrange("(f p) m -> p f m", p=128))
            for c in range(2):
                for f in range(2):
                    nc.sync.dma_start(out=xt[:, f, c * 512:(c + 1) * 512],
                                      in_=xTr[:, f, c * 512:(c + 1) * 512])
            twv = pers.tile([128, 2, 64], R, tag="twv")
            nc.sync.dma_start(out=twv[:], in_=wv.rearrange("(f p) m -> p f m", p=128))
            twg = pers.tile([128, 2, 64], R, tag="twg")
            nc.sync.dma_start(out=twg[:], in_=wgc.rearrange("(f p) m -> p f m", p=128))
            for c in range(2, 4):
                for f in range(2):
                    nc.sync.dma_start(out=xt[:, f, c * 512:(c + 1) * 512],
                                      in_=xTr[:, f, c * 512:(c + 1) * 512])
            tout = pers.tile([32, 2, 2, 128], R, tag="tout")
            nc.sync.dma_start(out=tout[:], in_=wout[:])
            tid = pers.tile([128, 128], F, tag="tid")
            nc.sync.dma_start(out=tid[:], in_=ident[:])
            tlm = pers.tile([128, 32], F, tag="tlm")
            nc.sync.dma_start(out=tlm[:], in_=lmask[:])
            tgam = pers.tile([32, 1], F, tag="tgam")
            nc.sync.dma_start(out=tgam[:], in_=gam[:])
            tbet = pers.tile([32, 1], F, tag="tbet")
            nc.sync.dma_start(out=tbet[:], in_=bet[:])
            tnbg = pers.tile([32, 2], F, tag="tnbg")
            nc.sync.dma_start(out=tnbg[:], in_=nbg[:])
            tpbg = pers.tile([32, 2], F, tag="tpbg")
            nc.sync.dma_start(out=tpbg[:], in_=pbg[:])
            teps = pers.tile([128, 1], F, tag="teps")
            nc.sync.dma_start(out=teps[:], in_=epsc[:])
            tones = pers.tile([128, 1], BF, tag="tones")
            nc.vector.memset(tones[:], 1.0)

            # ---------------- projections ----------------
            # q^T / k^T packed: partition 32d+j (j<16) = channel j of dhead d
            qTp = pers.tile([128, N], R, tag="qTp")
            kTp = pers.tile([128, N], R, tag="kTp")
            vpp = pers.tile([128, 16, 64], BF, tag="vpp")
            sge = pers.tile([32, 4, 1024], F, tag="sge")
            sgq = pers.tile([32, 4, 1024], F, tag="sgq")

            def proj_qk(dst, w, it):
                ps = dp.tile([128, 1024], F, tag="dots")
                for f in range(2):
                    nc.tensor.matmul(ps[:, 0:512], w[:, f, :],
                                     xt[:, f, it * 512:(it + 1) * 512],
                                     start=(f == 0), stop=(f == 1))
                nc.vector.tensor_copy(dst[:, it * 512:(it + 1) * 512], ps[:, 0:512])

            def proj_v(jc):
                ps = dp.tile([128, 1024], F, tag="dots")
                for f in range(2):
                    nc.tensor.matmul(ps[:, 0:64], xt[:, f, jc * 128:(jc + 1) * 128],
                                     twv[:, f, :], start=(f == 0), stop=(f == 1))
                nc.vector.tensor_copy(vpp[:, jc, :], ps[:, 0:64])

            def gates_mm(ip, b):
                ps = dp.tile([128, 1024], F, tag="dots")
                for nt in range(2):
                    for f in range(2):
                        nc.tensor.matmul(
                            ps[0:32, nt * 512:(nt + 1) * 512],
                            twg[:, f, 32 * b:32 * b + 32],
                            xt[:, f, ip * 1024 + nt * 512:ip * 1024 + (nt + 1) * 512],
                            start=(f == 0), stop=(f == 1))
                nc.scalar.activation(sge[:, 2 * ip + b, :], ps[0:32, :], AF.Exp,
                                     scale=-1.0, bias=tnbg[:, b:b + 1])

            proj_qk(kTp, twkp, 0)
            proj_qk(qTp, twqp, 0)
            proj_qk(qTp, twqp, 1)
            proj_v(0)

            def proj_rest():
                items = [lambda: proj_v(1),
                         lambda: proj_v(2), lambda: proj_v(3),
                         lambda: proj_qk(kTp, twkp, 1), lambda: proj_v(4), lambda: proj_v(5),
                         lambda: proj_qk(kTp, twkp, 2), lambda: proj_v(6), lambda: proj_v(7),
                         lambda: proj_qk(kTp, twkp, 3), lambda: proj_v(8), lambda: proj_v(9),
                         lambda: proj_qk(qTp, twqp, 2), lambda: proj_v(10), lambda: proj_v(11),
                         lambda: proj_qk(qTp, twqp, 3), lambda: proj_v(12), lambda: proj_v(13),
                         lambda: proj_v(14), lambda: proj_v(15),
                         lambda: gates_mm(0, 0), lambda: gates_mm(0, 1)]
                for i, it in enumerate(items):
                    it()
                    if i >= 10 or i % 2 == 1:
                        yield

            # ---------------- attention ----------------
            saved = {}

            def attention(ip, interleave=None):
                i0 = ip * 1024
                zreg = zp.tile([128, 32, 32], F, tag="zreg")
                sreg = spp.tile([128, 32], F, tag="sreg")

                def zs_mm(jc, ebs):
                    # z/s matmuls lag the dots of the NEXT jc so the PE never
                    # blocks on the current jc's last exp
                    for dh in range(4):
                        pair = dh // 2
                        eb = ebs[dh]
                        for ib in range(8):
                            slot = ib * 4 + dh
                            st = eb[:, ib * 128:(ib + 1) * 128]
                            nc.tensor.matmul(
                                zreg[:, slot, :], st,
                                vpp[:, jc, 32 * pair:32 * pair + 32],
                                start=(jc == 0 and slot in (0, 16)),
                                stop=(jc == 15),
                                skip_group_check=True,
                            )
                            nc.tensor.matmul(
                                sreg[:, slot:slot + 1], st, tones[:],
                                start=(jc == 0 and slot == 0),
                                stop=(jc == 15),
                                skip_group_check=True,
                            )

                prev = None
                for jc in range(16):
                    j0 = jc * 128
                    ebs = []
                    for dh in range(4):
                        r0 = 32 * dh
                        dt_ = dp.tile([128, 1024], F, tag="dots")
                        for h in range(2):
                            nc.tensor.matmul(
                                dt_[:, h * 512:(h + 1) * 512],
                                kTp[r0:r0 + 16, j0:j0 + 128],
                                qTp[r0:r0 + 16, i0 + h * 512:i0 + (h + 1) * 512],
                                start=True, stop=True,
                                tile_position=(r0, 0),
                            )
                        eb = ebp.tile([128, 1024], BF, tag="ebf")
                        nc.scalar.activation(eb[:], dt_[:], AF.Exp)
                        ebs.append(eb)
                    if prev is not None:
                        zs_mm(jc - 1, prev)
                    prev = ebs
                    if interleave is not None:
                        next(interleave, None)
                zs_mm(15, prev)
                saved[ip] = (zreg, sreg)

            def epi_dve(ip):
                """u/LN chain in [i, c] domain, split in two pipelined halves
                (pairslots 0:8 | 8:16 = slots 0:16 | 16:32)."""
                zreg, sreg = saved[ip]
                ucs = []
                for hb in range(2):
                    s0, ps0 = 16 * hb, 8 * hb
                    sfx = f"{ip}{hb}"
                    rinv = ep.tile([128, 16], F, tag=f"rinv{hb}", name=f"rinv{sfx}")
                    nc.vector.reciprocal(rinv[:], sreg[:, s0:s0 + 16])
                    rinl = ep.tile([128, 16], F, tag=f"rinl{hb}", name=f"rinl{sfx}")
                    nc.vector.tensor_tensor(rinl[:], rinv[:], tlm[:, s0:s0 + 16], AL.mult)
                    zr = ep.tile([128, 16, 32], F, tag=f"zr{hb}", name=f"zr{sfx}")
                    nc.vector.tensor_tensor(zr[:], zreg[:, s0:s0 + 16, :],
                                            _bcast(rinl[:], 32), AL.mult)
                    u = ep.tile([128, 8, 32], F, tag=f"u{hb}", name=f"u{sfx}")
                    nc.vector.tensor_tensor(u[:], zr[:, 0:16:2, :], zr[:, 1:16:2, :],
                                            AL.subtract)
                    usq = ep.tile([128, 8, 32], F, tag=f"usq{hb}", name=f"usq{sfx}")
                    nc.vector.tensor_tensor(usq[:], u[:], u[:], AL.mult)
                    s1 = ep.tile([128, 8], F, tag=f"s1{hb}", name=f"s1{sfx}")
                    nc.vector.tensor_reduce(s1[:], u[:], AX.X, AL.add)
                    s2 = ep.tile([128, 8], F, tag=f"s2{hb}", name=f"s2{sfx}")
                    nc.vector.tensor_reduce(s2[:], usq[:], AX.X, AL.add)
                    mu = ep.tile([128, 8], F, tag=f"mu{hb}", name=f"mu{sfx}")
                    nc.vector.tensor_scalar(mu[:], s1[:], 1.0 / 32.0, None, AL.mult)
                    var = ep.tile([128, 8], F, tag=f"var{hb}", name=f"var{sfx}")
                    nc.vector.tensor_tensor(var[:], mu[:], mu[:], AL.mult)
                    nc.vector.tensor_scalar(var[:], var[:], 32.0, None, AL.mult)
                    nc.vector.tensor_tensor(var[:], s2[:], var[:], AL.subtract)
                    w = ep.tile([128, 8], F, tag=f"w{hb}", name=f"w{sfx}")
                    nc.scalar.activation(var[:], var[:], AF.Ln, scale=1.0 / 32.0,
                                         bias=teps[:])
                    nc.scalar.activation(w[:], var[:], AF.Exp, scale=-0.5)
                    uc = ep.tile([128, 8, 32], F, tag=f"uc{hb}", name=f"uc{sfx}")
                    nc.vector.tensor_tensor(uc[:], u[:], _bcast(mu[:], 32), AL.subtract)
                    nc.vector.tensor_tensor(uc[:], uc[:], _bcast(w[:], 32), AL.mult)
                    ucs.append(uc)
                saved[ip] = ucs

            def epi_out(ip):
                """u'^T via PE transposes, gamma/beta + gating on DVE, output
                projection per 512-col chunk; yields between chunks."""
                ucs = saved[ip]
                gr2 = ep.tile([32, 2, 1024], R, tag="gr2", name=f"gr2_{ip}")
                # one psum bank holding two manual chunk buffers (region-level
                # dep tracking pipelines transposes against the gating mults)
                tpt = tpp.tile([32, 2, 2, 128], F, tag="tpt", name=f"tpt_{ip}")
                for nt in range(2):
                    for ib4 in range(4):
                        ib = nt * 4 + ib4
                        bb = ib % 2
                        for pair in range(2):
                            psl = (ib * 2 + pair) % 8
                            nc.tensor.transpose(tpt[:, bb, pair, :],
                                                ucs[ib // 4][:, psl, :], tid[:])
                        c0 = ib * 128
                        if beta_zero:
                            # gamma folded into wout on the host; beta==0:
                            # gating is a single mult straight off the psum
                            nc.vector.tensor_tensor(
                                gr2[:, :, c0:c0 + 128], tpt[:, bb, :, :],
                                sgq[:, 2 * ip:2 * ip + 2, c0:c0 + 128], AL.mult)
                        else:
                            gg = ep.tile([32, 2, 128], F, tag="gg",
                                         name=f"gg_{ip}{ib}")
                            nc.vector.tensor_scalar(gg[:], tpt[:, bb, :, :], tgam[:],
                                                    tbet[:], AL.mult, AL.add)
                            nc.vector.tensor_tensor(
                                gr2[:, :, c0:c0 + 128], gg[:],
                                sgq[:, 2 * ip:2 * ip + 2, c0:c0 + 128], AL.mult)
                        if ib % 2 == 1:
                            yield
                    yp = dp.tile([128, 1024], F, tag="dots")
                    for oh in range(2):
                        for pair in range(2):
                            nc.tensor.matmul(yp[:, oh * 512:(oh + 1) * 512],
                                             tout[:, pair, oh, :],
                                             gr2[:, pair, nt * 512:(nt + 1) * 512],
                                             start=(pair == 0), stop=(pair == 1))
                    ys = ep.tile([128, 1024], F, tag=f"ys{nt}", name=f"ys{nt}_{ip}")
                    if ip == 1 and nt == 0:
                        # keep DVE free for the ongoing epilogue cascade:
                        # copies via idle ACT
                        nc.scalar.copy(ys[:, 0:512], yp[:, 0:512])
                        nc.scalar.copy(ys[:, 512:1024], yp[:, 512:1024])
                    elif ip == 1:
                        # last chunk: split across ACT and DVE
                        nc.scalar.copy(ys[:, 0:512], yp[:, 0:512])
                        nc.vector.tensor_copy(ys[:, 512:1024], yp[:, 512:1024])
                    else:
                        nc.vector.tensor_copy(ys[:], yp[:])
                    c0 = ip * 1024 + nt * 512
                    nc.sync.dma_start(out=yT[0:128, c0:c0 + 512], in_=ys[:, 0:512])
                    eng = nc.scalar if ip == 1 else nc.sync
                    eng.dma_start(out=yT[128:256, c0:c0 + 512], in_=ys[:, 512:1024])
                    yield

            gen_proj = proj_rest()
            attention(0, interleave=gen_proj)
            for _ in gen_proj:
                pass
            # finish sigmoid for ip0 off the critical engines
            nc.gpsimd.tensor_scalar(sge[:, 0:2, :], sge[:, 0:2, :], 1.0, None, AL.add)
            nc.vector.reciprocal(sgq[:, 0:2, :], sge[:, 0:2, :])
            epi_dve(0)
            gen0 = epi_out(0)

            class InterleaveAt:
                """Step gen0 only during the later jc's of attention(1)."""
                def __init__(self, gen, start):
                    self.gen, self.start, self.jc = gen, start, 0
                def __next__(self):
                    if self.jc >= self.start:
                        next(self.gen, None)
                    self.jc += 1
                    return None

            attention(1, interleave=InterleaveAt(gen0, 8))
            for _ in gen0:
                pass
            # gates for ip1: the whole sigmoid runs on ACT inside the
            # post-attention ACT gap: sig = exp(-ln(exp(-(g+bg)) + 1))
            gates_mm(1, 0)
            gates_mm(1, 1)
            nc.scalar.activation(sge[:, 2:4, :], sge[:, 2:4, :], AF.Ln, bias=1.0)
            nc.scalar.activation(sgq[:, 2:4, :], sge[:, 2:4, :], AF.Exp, scale=-1.0)
            epi_dve(1)
            for _ in epi_out(1):
                pass

    nc.finalize()
    return nc


def _prep_core_inputs(inputs, bi, hg, lam, beta_zero=True):
    scale = DH ** -0.5
    x = np.asarray(inputs["x"], np.float32)
    Wq = np.asarray(inputs["Wq"], np.float32)
    Wkv = np.asarray(inputs["Wkv"], np.float32)
    Wout = np.asarray(inputs["Wout"], np.float32)
    Wg = np.asarray(inputs["Wg"], np.float32)
    bg = np.asarray(inputs["bg"], np.float32)
    g_ = np.asarray(inputs["ln_gamma"], np.float32)
    b_ = np.asarray(inputs["ln_beta"], np.float32)
    li = np.float32(1.0 - LAMBDA_INIT)

    c0 = 64 * hg
    wq_c = Wq[:, c0:c0 + 64] * scale
    wk_c = Wkv[:, c0:c0 + 64]
    wv_c = Wkv[:, 256 + c0:256 + c0 + 64]
    wg_c = Wg[:, c0:c0 + 64]
    wout_c = Wout[c0:c0 + 64, :]

    wqp = np.zeros((256, 128), np.float32)
    wkp = np.zeros((256, 128), np.float32)
    for d in range(4):
        wqp[:, 32 * d:32 * d + 16] = wq_c[:, 16 * d:16 * d + 16]
        wkp[:, 32 * d:32 * d + 16] = wk_c[:, 16 * d:16 * d + 16]

    gvec = (g_[0:32] * li).astype(np.float32)
    woutp = np.zeros((32, 2, 2, 128), np.float32)
    for pair in range(2):
        for oh in range(2):
            woutp[:, pair, oh, :] = wout_c[32 * pair:32 * pair + 32,
                                           128 * oh:128 * oh + 128]
            if beta_zero:
                # gamma (and the 1-lambda_init factor) folded into wout
                woutp[:, pair, oh, :] *= gvec[:, None]

    lmask = np.ones((128, 32), np.float32)
    for slot in range(32):
        if slot % 4 in (1, 3):
            lmask[:, slot] = lam

    nbg = np.zeros((32, 2), np.float32)
    pbg = np.zeros((32, 2), np.float32)
    for b in range(2):
        nbg[:, b] = -bg[c0 + 32 * b:c0 + 32 * b + 32]
        pbg[:, b] = bg[c0 + 32 * b:c0 + 32 * b + 32]

    return {
        "xT": np.ascontiguousarray(x[bi].T),
        "wqp": wqp, "wkp": wkp,
        "wv": np.ascontiguousarray(wv_c),
        "wgc": np.ascontiguousarray(wg_c),
        "wout": woutp,
        "ident": np.eye(128, dtype=np.float32),
        "lmask": lmask,
        "gam": (g_[0:32] * li).reshape(32, 1).astype(np.float32),
        "bet": (b_[0:32] * li).reshape(32, 1).astype(np.float32),
        "nbg": nbg, "pbg": pbg,
        "epsc": np.full((128, 1), LN_EPS, np.float32),
    }


def kernel(**inputs) -> np.ndarray:
    lq1 = np.asarray(inputs["lq1"], np.float64)
    lk1 = np.asarray(inputs["lk1"], np.float64)
    lq2 = np.asarray(inputs["lq2"], np.float64)
    lk2 = np.asarray(inputs["lk2"], np.float64)
    lam = float(np.exp(np.sum(lq1 * lk1)) - np.exp(np.sum(lq2 * lk2)) + LAMBDA_INIT)
    bout = np.asarray(inputs["bout"], np.float32)
    beta_zero = bool(np.all(np.asarray(inputs["ln_beta"]) == 0.0))

    key = ("nc", beta_zero)
    if key not in _cached:
        _cached[key] = build_kernel(beta_zero)
    nc = _cached[key]

    in_maps = []
    for c in range(NC):
        bi, hg = c // 4, c % 4
        in_maps.append(_prep_core_inputs(inputs, bi, hg, lam, beta_zero))

    import os
    trace = bool(int(os.environ.get("BASS_KERNEL_TRACE", "0")))
    res = run_bass_kernel_spmd(nc, in_maps, list(range(NC)), trace=trace)
    _cached["exec_time_ns"] = res.exec_time_ns
    _cached["trace"] = res.instructions_and_trace
    out = np.zeros((B, N, DIM), np.float32)
    for c in range(NC):
        bi = c // 4
        out[bi] += res.results[c]["yT"].T
    out += bout
    return out


# revision 42
# speedup vs baseline: 1.0049x; 1.0000x over previous
"""DiffAttn2d TRN2 Bass kernel (v2).

Sharding: 8 cores = 2 (batch) x 4 (head-groups of 2 heads / 4 doubled-heads).

Per core (channel-major scores [key j, query i], n=2048, 4 doubled heads):
  - dots^T via row-packed K=16 fp32r matmuls (4 dheads in 4 PE row groups)
  - one ACT exp pass per (jc, dh) dots tile PSUM->SBUF bf16 (the only O(n^2)
    elementwise op; ACT is the bottleneck engine at ~133us busy)
  - attn @ v with the EXP TILE STATIONARY: out z^T[i, c] = sum_j exp[j,i] v[j,c]
    -> 32-row matmuls (cost model charges moving/out rows only).  32 slot
    accumulators interleave in 2 PSUM banks; a single bank-starting matmul
    (jc==0, slot 0/16) lazily zeroes the bank, remaining slots' first writes
    init via the pending-zero flag.  Softmax denominators from 1-row
    ones-matmuls accumulated in [128, 32] PSUM the same way.
  - epilogue in [i-part, c-free] domain on DVE (two pipelined halves):
    reciprocal, lambda-fold, stride-0 broadcast mults, strided even-odd
    subtract, free-dim reduces for LN stats; rsqrt via tiny [128, 8] Ln/Exp
    on ACT (zero table switches)
  - PE f32 transposes (identity moving) bring u' back to [c, i] for
    gamma/beta, sigmoid gating, and the K=32 output projection
  - gates: sigmoid = 1/(1+e^-g): ACT only does e^-g; +1 on Pool; reciprocal
    on DVE for ip0 and ones-divide on Pool for ip1 (tail, off both
    bottleneck engines)
  - y^T partials summed on host (+ bout)
"""
import sys
sys.path.insert(0, "/opt/trn_rl_repo")

import math
import numpy as np
import ml_dtypes

import concourse.bass as bass
import concourse.bacc as bacc_mod
import concourse.mybir as mybir
from concourse.tile import TileContext
from concourse.bass_utils import run_bass_kernel_spmd

F = mybir.dt.float32
R = mybir.dt.float32r
BF = mybir.dt.bfloat16
AF = mybir.ActivationFunctionType
AL = mybir.AluOpType
AX = mybir.AxisListType

H, DH = 8, 16
DEPTH = 1
LAMBDA_INIT = 0.8 - 0.6 * math.exp(-0.3 * DEPTH)
LN_EPS = 1e-5
B, N, DIM = 2, 2048, 256
NC = 8

_cached = {}


def _bcast(ap, n):
    """Append a stride-0 free dim of size n (broadcast along free)."""
    return bass.AP(ap.tensor, ap.offset, list(ap.ap) + [[0, n]])


def build_kernel(beta_zero=True):
    nc = bacc_mod.Bacc()
    xT = nc.declare_dram_parameter("xT", [DIM, N], R, isOutput=False)
    wqp = nc.declare_dram_parameter("wqp", [DIM, 128], R, isOutput=False)
    wkp = nc.declare_dram_parameter("wkp", [DIM, 128], R, isOutput=False)
    wv = nc.declare_dram_parameter("wv", [DIM, 64], R, isOutput=False)
    wgc = nc.declare_dram_parameter("wgc", [DIM, 64], R, isOutput=False)
    wout = nc.declare_dram_parameter("wout", [32, 2, 2, 128], R, isOutput=False)
    ident = nc.declare_dram_parameter("ident", [128, 128], F, isOutput=False)
    lmask = nc.declare_dram_parameter("lmask", [128, 32], F, isOutput=False)
    gam = nc.declare_dram_parameter("gam", [32, 1], F, isOutput=False)
    bet = nc.declare_dram_parameter("bet", [32, 1], F, isOutput=False)
    nbg = nc.declare_dram_parameter("nbg", [32, 2], F, isOutput=False)
    pbg = nc.declare_dram_parameter("pbg", [32, 2], F, isOutput=False)
    epsc = nc.declare_dram_parameter("epsc", [128, 1], F, isOutput=False)
    yT = nc.declare_dram_parameter("yT", [DIM, N], F, isOutput=True)

    with TileContext(nc) as tc:
        with tc.tile_pool(name="pers", bufs=1) as pers, \
             tc.tile_pool(name="ebp", bufs=12) as ebp, \
             tc.tile_pool(name="ep", bufs=2) as ep, \
             tc.tile_pool(name="dp", bufs=2, space="PSUM") as dp, \
             tc.tile_pool(name="zp", bufs=1, space="PSUM") as zp, \
             tc.tile_pool(name="tp", bufs=1, space="PSUM") as tpp, \
             tc.tile_pool(name="sp", bufs=1, space="PSUM") as spp:

            # prewarm ACT exp/ln table set
            warm = pers.tile([1, 8], F, tag="warm")
            nc.vector.memset(warm[:], 0.0)
            nc.scalar.activation(warm[:], warm[:], AF.Exp)

            # ---------------- DMAs (ordered for earliest projections) -------
            xt = pers.tile([128, 2, N], R, tag="xt")
            xTr = xT.rearrange("(f p) n -> p f n", p=128)
            twkp = pers.tile([128, 2, 128], R, tag="twkp")
            nc.sync.dma_start(out=twkp[:], in_=wkp.rearrange("(f p) m -> p f m", p=128))
            twqp = pers.tile([128, 2, 128], R, tag="twqp")
            nc.sync.dma_start(out=twqp[:], in_=wqp.rearrange("(f p) m -> p f m", p=128))
            for c in range(2):
                for f in range(2):
                    nc.sync.dma_start(out=xt[:, f, c * 512:(c + 1) * 512],
                                      in_=xTr[:, f, c * 512:(c + 1) * 512])
            twv = pers.tile([128, 2, 64], R, tag="twv")
            nc.sync.dma_start(out=twv[:], in_=wv.rearrange("(f p) m -> p f m", p=128))
            twg = pers.tile([128, 2, 64], R, tag="twg")
            nc.sync.dma_start(out=twg[:], in_=wgc.rearrange("(f p) m -> p f m", p=128))
            for c in range(2, 4):
                for f in range(2):
                    nc.sync.dma_start(out=xt[:, f, c * 512:(c + 1) * 512],
                                      in_=xTr[:, f, c * 512:(c + 1) * 512])
            tout = pers.tile([32, 2, 2, 128], R, tag="tout")
            nc.sync.dma_start(out=tout[:], in_=wout[:])
            tid = pers.tile([128, 128], F, tag="tid")
            nc.sync.dma_start(out=tid[:], in_=ident[:])
            tlm = pers.tile([128, 32], F, tag="tlm")
            nc.sync.dma_start(out=tlm[:], in_=lmask[:])
            tgam = pers.tile([32, 1], F, tag="tgam")
            nc.sync.dma_start(out=tgam[:], in_=gam[:])
            tbet = pers.tile([32, 1], F, tag="tbet")
            nc.sync.dma_start(out=tbet[:], in_=bet[:])
            tnbg = pers.tile([32, 2], F, tag="tnbg")
            nc.sync.dma_start(out=tnbg[:], in_=nbg[:])
            tpbg = pers.tile([32, 2], F, tag="tpbg")
            nc.sync.dma_start(out=tpbg[:], in_=pbg[:])
            teps = pers.tile([128, 1], F, tag="teps")
            nc.sync.dma_start(out=teps[:], in_=epsc[:])
            tones = pers.tile([128, 1], BF, tag="tones")
            nc.vector.memset(tones[:], 1.0)

            # ---------------- projections ----------------
            # q^T / k^T packed: partition 32d+j (j<16) = channel j of dhead d
            qTp = pers.tile([128, N], R, tag="qTp")
            kTp = pers.tile([128, N], R, tag="kTp")
            vpp = pers.tile([128, 16, 64], BF, tag="vpp")
            sge = pers.tile([32, 4, 1024], F, tag="sge")
            sgq = pers.tile([32, 4, 1024], F, tag="sgq")

            def proj_qk(dst, w, it):
                ps = dp.tile([128, 1024], F, tag="dots")
                for f in range(2):
                    nc.tensor.matmul(ps[:, 0:512], w[:, f, :],
                                     xt[:, f, it * 512:(it + 1) * 512],
                                     start=(f == 0), stop=(f == 1))
                nc.vector.tensor_copy(dst[:, it * 512:(it + 1) * 512], ps[:, 0:512])

            def proj_v(jc):
                ps = dp.tile([128, 1024], F, tag="dots")
                for f in range(2):
                    nc.tensor.matmul(ps[:, 0:64], xt[:, f, jc * 128:(jc + 1) * 128],
                                     twv[:, f, :], start=(f == 0), stop=(f == 1))
                nc.vector.tensor_copy(vpp[:, jc, :], ps[:, 0:64])

            def gates_mm(ip, b):
                ps = dp.tile([128, 1024], F, tag="dots")
                for nt in range(2):
                    for f in range(2):
                        nc.tensor.matmul(
                            ps[0:32, nt * 512:(nt + 1) * 512],
                            twg[:, f, 32 * b:32 * b + 32],
                            xt[:, f, ip * 1024 + nt * 512:ip * 1024 + (nt + 1) * 512],
                            start=(f == 0), stop=(f == 1))
                nc.scalar.activation(sge[:, 2 * ip + b, :], ps[0:32, :], AF.Exp,
                                     scale=-1.0, bias=tnbg[:, b:b + 1])

            proj_qk(kTp, twkp, 0)
            proj_qk(qTp, twqp, 0)
            proj_qk(qTp, twqp, 1)
            proj_v(0)

            def proj_rest():
                items = [lambda: proj_v(1),
                         lambda: proj_v(2), lambda: proj_v(3),
                         lambda: proj_qk(kTp, twkp, 1), lambda: proj_v(4), lambda: proj_v(5),
                         lambda: proj_qk(kTp, twkp, 2), lambda: proj_v(6), lambda: proj_v(7),
                         lambda: proj_qk(kTp, twkp, 3), lambda: proj_v(8), lambda: proj_v(9),
                         lambda: proj_qk(qTp, twqp, 2), lambda: proj_v(10), lambda: proj_v(11),
                         lambda: proj_qk(qTp, twqp, 3), lambda: proj_v(12), lambda: proj_v(13),
                         lambda: proj_v(14), lambda: proj_v(15),
                         lambda: gates_mm(0, 0), lambda: gates_mm(0, 1)]
                for i, it in enumerate(items):
                    it()
                    if i >= 10 or i % 2 == 1:
                        yield

            # ---------------- attention ----------------
            saved = {}

            def attention(ip, interleave=None):
                i0 = ip * 1024
                zreg = zp.tile([128, 32, 32], F, tag="zreg")
                sreg = spp.tile([128, 32], F, tag="sreg")

                def zs_mm(jc, ebs):
                    # z/s matmuls lag the dots of the NEXT jc so the PE never
                    # blocks on the current jc's last exp
                    for dh in range(4):
                        pair = dh // 2
                        eb = ebs[dh]
                        for ib in range(8):
                            slot = ib * 4 + dh
                            st = eb[:, ib * 128:(ib + 1) * 128]
                            nc.tensor.matmul(
                                zreg[:, slot, :], st,
                                vpp[:, jc, 32 * pair:32 * pair + 32],
                                start=(jc == 0 and slot in (0, 16)),
                                stop=(jc == 15),
                                skip_group_check=True,
                            )
                            nc.tensor.matmul(
                                sreg[:, slot:slot + 1], st, tones[:],
                                start=(jc == 0 and slot == 0),
                                stop=(jc == 15),
                                skip_group_check=True,
                            )

                prev = None
                for jc in range(16):
                    if interleave is not None:
                        next(interleave, None)
                    j0 = jc * 128
                    ebs = []
                    for dh in range(4):
                        r0 = 32 * dh
                        dt_ = dp.tile([128, 1024], F, tag="dots")
                        for h in range(2):
                            nc.tensor.matmul(
                                dt_[:, h * 512:(h + 1) * 512],
                                kTp[r0:r0 + 16, j0:j0 + 128],
                                qTp[r0:r0 + 16, i0 + h * 512:i0 + (h + 1) * 512],
                                start=True, stop=True,
                                tile_position=(r0, 0),
                            )
                        eb = ebp.tile([128, 1024], BF, tag="ebf")
                        nc.scalar.activation(eb[:], dt_[:], AF.Exp)
                        ebs.append(eb)
                    if prev is not None:
                        zs_mm(jc - 1, prev)
                    prev = ebs
                zs_mm(15, prev)
                saved[ip] = (zreg, sreg)

            def epi_dve(ip):
                """u/LN chain in [i, c] domain, split in two pipelined halves
                (pairslots 0:8 | 8:16 = slots 0:16 | 16:32)."""
                zreg, sreg = saved[ip]
                ucs = []
                for hb in range(2):
                    s0, ps0 = 16 * hb, 8 * hb
                    sfx = f"{ip}{hb}"
                    rinv = ep.tile([128, 16], F, tag=f"rinv{hb}", name=f"rinv{sfx}")
                    nc.vector.reciprocal(rinv[:], sreg[:, s0:s0 + 16])
                    rinl = ep.tile([128, 16], F, tag=f"rinl{hb}", name=f"rinl{sfx}")
                    nc.vector.tensor_tensor(rinl[:], rinv[:], tlm[:, s0:s0 + 16], AL.mult)
                    zr = ep.tile([128, 16, 32], F, tag=f"zr{hb}", name=f"zr{sfx}")
                    nc.vector.tensor_tensor(zr[:], zreg[:, s0:s0 + 16, :],
                                            _bcast(rinl[:], 32), AL.mult)
                    u = ep.tile([128, 8, 32], F, tag=f"u{hb}", name=f"u{sfx}")
                    nc.vector.tensor_tensor(u[:], zr[:, 0:16:2, :], zr[:, 1:16:2, :],
                                            AL.subtract)
                    usq = ep.tile([128, 8, 32], F, tag=f"usq{hb}", name=f"usq{sfx}")
                    nc.vector.tensor_tensor(usq[:], u[:], u[:], AL.mult)
                    s1 = ep.tile([128, 8], F, tag=f"s1{hb}", name=f"s1{sfx}")
                    nc.vector.tensor_reduce(s1[:], u[:], AX.X, AL.add)
                    s2 = ep.tile([128, 8], F, tag=f"s2{hb}", name=f"s2{sfx}")
                    nc.vector.tensor_reduce(s2[:], usq[:], AX.X, AL.add)
                    mu = ep.tile([128, 8], F, tag=f"mu{hb}", name=f"mu{sfx}")
                    nc.vector.tensor_scalar(mu[:], s1[:], 1.0 / 32.0, None, AL.mult)
                    var = ep.tile([128, 8], F, tag=f"var{hb}", name=f"var{sfx}")
                    nc.vector.tensor_tensor(var[:], mu[:], mu[:], AL.mult)
                    nc.vector.tensor_scalar(var[:], var[:], 32.0, None, AL.mult)
                    nc.vector.tensor_tensor(var[:], s2[:], var[:], AL.subtract)
                    w = ep.tile([128, 8], F, tag=f"w{hb}", name=f"w{sfx}")
                    nc.scalar.activation(var[:], var[:], AF.Ln, scale=1.0 / 32.0,
                                         bias=teps[:])
                    nc.scalar.activation(w[:], var[:], AF.Exp, scale=-0.5)
                    uc = ep.tile([128, 8, 32], F, tag=f"uc{hb}", name=f"uc{sfx}")
                    nc.vector.tensor_tensor(uc[:], u[:], _bcast(mu[:], 32), AL.subtract)
                    nc.vector.tensor_tensor(uc[:], uc[:], _bcast(w[:], 32), AL.mult)
                    ucs.append(uc)
                saved[ip] = ucs

            def epi_out(ip):
                """u'^T via PE transposes, gamma/beta + gating on DVE, output
                projection per 512-col chunk; yields between chunks."""
                ucs = saved[ip]
                gr2 = ep.tile([32, 2, 1024], R, tag="gr2", name=f"gr2_{ip}")
                # one psum bank holding two manual chunk buffers (region-level
                # dep tracking pipelines transposes against the gating mults)
                tpt = tpp.tile([32, 2, 2, 128], F, tag="tpt", name=f"tpt_{ip}")
                for nt in range(2):
                    for ib4 in range(4):
                        ib = nt * 4 + ib4
                        bb = ib % 2
                        for pair in range(2):
                            psl = (ib * 2 + pair) % 8
                            nc.tensor.transpose(tpt[:, bb, pair, :],
                                                ucs[ib // 4][:, psl, :], tid[:])
                        c0 = ib * 128
                        if beta_zero:
                            # gamma folded into wout on the host; beta==0:
                            # gating is a single mult straight off the psum
                            nc.vector.tensor_tensor(
                                gr2[:, :, c0:c0 + 128], tpt[:, bb, :, :],
                                sgq[:, 2 * ip:2 * ip + 2, c0:c0 + 128], AL.mult)
                        else:
                            gg = ep.tile([32, 2, 128], F, tag="gg",
                                         name=f"gg_{ip}{ib}")
                            nc.vector.tensor_scalar(gg[:], tpt[:, bb, :, :], tgam[:],
                                                    tbet[:], AL.mult, AL.add)
                            nc.vector.tensor_tensor(
                                gr2[:, :, c0:c0 + 128], gg[:],
                                sgq[:, 2 * ip:2 * ip + 2, c0:c0 + 128], AL.mult)
                        if ib % 2 == 1:
                            yield
                    yp = dp.tile([128, 1024], F, tag="dots")
                    for oh in range(2):
                        for pair in range(2):
                            nc.tensor.matmul(yp[:, oh * 512:(oh + 1) * 512],
                                             tout[:, pair, oh, :],
                                             gr2[:, pair, nt * 512:(nt + 1) * 512],
                                             start=(pair == 0), stop=(pair == 1))
                    ys = ep.tile([128, 1024], F, tag=f"ys{nt}", name=f"ys{nt}_{ip}")
                    if ip == 1 and nt == 0:
                        # keep DVE free for the ongoing epilogue cascade:
                        # copies via idle ACT
                        nc.scalar.copy(ys[:, 0:512], yp[:, 0:512])
                        nc.scalar.copy(ys[:, 512:1024], yp[:, 512:1024])
                    elif ip == 1:
                        # last chunk: split across ACT and DVE
                        nc.scalar.copy(ys[:, 0:512], yp[:, 0:512])
                        nc.vector.tensor_copy(ys[:, 512:1024], yp[:, 512:1024])
                    else:
                        nc.vector.tensor_copy(ys[:], yp[:])
                    c0 = ip * 1024 + nt * 512
                    nc.sync.dma_start(out=yT[0:128, c0:c0 + 512], in_=ys[:, 0:512])
                    eng = nc.scalar if ip == 1 else nc.sync
                    eng.dma_start(out=yT[128:256, c0:c0 + 512], in_=ys[:, 512:1024])
                    yield

            gen_proj = proj_rest()
            attention(0, interleave=gen_proj)
            for _ in gen_proj:
                pass
            # finish sigmoid for ip0 off the critical engines
            nc.gpsimd.tensor_scalar(sge[:, 0:2, :], sge[:, 0:2, :], 1.0, None, AL.add)
            nc.vector.reciprocal(sgq[:, 0:2, :], sge[:, 0:2, :])
            epi_dve(0)
            gen0 = epi_out(0)

            class InterleaveAt:
                """Step gen0 only during the later jc's of attention(1)."""
                def __init__(self, gen, start):
                    self.gen, self.start, self.jc = gen, start, 0
                def __next__(self):
                    if self.jc >= self.start:
                        next(self.gen, None)
                    self.jc += 1
                    return None

            attention(1, interleave=InterleaveAt(gen0, 8))
            for _ in gen0:
                pass
            # gates for ip1: the whole sigmoid runs on ACT inside the
            # post-attention ACT gap: sig = exp(-ln(exp(-(g+bg)) + 1))
            gates_mm(1, 0)
            gates_mm(1, 1)
            nc.scalar.activation(sge[:, 2:4, :], sge[:, 2:4, :], AF.Ln, bias=1.0)
            nc.scalar.activation(sgq[:, 2:4, :], sge[:, 2:4, :], AF.Exp, scale=-1.0)
            epi_dve(1)
            for _ in epi_out(1):
                pass

    nc.finalize()
    return nc


def _prep_core_inputs(inputs, bi, hg, lam, beta_zero=True):
    scale = DH ** -0.5
    x = np.asarray(inputs["x"], np.float32)
    Wq = np.asarray(inputs["Wq"], np.float32)
    Wkv = np.asarray(inputs["Wkv"], np.float32)
    Wout = np.asarray(inputs["Wout"], np.float32)
    Wg = np.asarray(inputs["Wg"], np.float32)
    bg = np.asarray(inputs["bg"], np.float32)
    g_ = np.asarray(inputs["ln_gamma"], np.float32)
    b_ = np.asarray(inputs["ln_beta"], np.float32)
    li = np.float32(1.0 - LAMBDA_INIT)

    c0 = 64 * hg
    wq_c = Wq[:, c0:c0 + 64] * scale
    wk_c = Wkv[:, c0:c0 + 64]
    wv_c = Wkv[:, 256 + c0:256 + c0 + 64]
    wg_c = Wg[:, c0:c0 + 64]
    wout_c = Wout[c0:c0 + 64, :]

    wqp = np.zeros((256, 128), np.float32)
    wkp = np.zeros((256, 128), np.float32)
    for d in range(4):
        wqp[:, 32 * d:32 * d + 16] = wq_c[:, 16 * d:16 * d + 16]
        wkp[:, 32 * d:32 * d + 16] = wk_c[:, 16 * d:16 * d + 16]

    gvec = (g_[0:32] * li).astype(np.float32)
    woutp = np.zeros((32, 2, 2, 128), np.float32)
    for pair in range(2):
        for oh in range(2):
            woutp[:, pair, oh, :] = wout_c[32 * pair:32 * pair + 32,
                                           128 * oh:128 * oh + 128]
            if beta_zero:
                # gamma (and the 1-lambda_init factor) folded into wout
                woutp[:, pair, oh, :] *= gvec[:, None]

    lmask = np.ones((128, 32), np.float32)
    for slot in range(32):
        if slot % 4 in (1, 3):
            lmask[:, slot] = lam

    nbg = np.zeros((32, 2), np.float32)
    pbg = np.zeros((32, 2), np.float32)
    for b in range(2):
        nbg[:, b] = -bg[c0 + 32 * b:c0 + 32 * b + 32]
        pbg[:, b] = bg[c0 + 32 * b:c0 + 32 * b + 32]

    return {
        "xT": np.ascontiguousarray(x[bi].T),
        "wqp": wqp, "wkp": wkp,
        "wv": np.ascontiguousarray(wv_c),
        "wgc": np.ascontiguousarray(wg_c),
        "wout": woutp,
        "ident": np.eye(128, dtype=np.float32),
        "lmask": lmask,
        "gam": (g_[0:32] * li).reshape(32, 1).astype(np.float32),
        "bet": (b_[0:32] * li).reshape(32, 1).astype(np.float32),
        "nbg": nbg, "pbg": pbg,
        "epsc": np.full((128, 1), LN_EPS, np.float32),
    }


def kernel(**inputs) -> np.ndarray:
    lq1 = np.asarray(inputs["lq1"], np.float64)
    lk1 = np.asarray(inputs["lk1"], np.float64)
    lq2 = np.asarray(inputs["lq2"], np.float64)
    lk2 = np.asarray(inputs["lk2"], np.float64)
    lam = float(np.exp(np.sum(lq1 * lk1)) - np.exp(np.sum(lq2 * lk2)) + LAMBDA_INIT)
    bout = np.asarray(inputs["bout"], np.float32)
    beta_zero = bool(np.all(np.asarray(inputs["ln_beta"]) == 0.0))

    key = ("nc", beta_zero)
    if key not in _cached:
        _cached[key] = build_kernel(beta_zero)
    nc = _cached[key]

    in_maps = []
    for c in range(NC):
        bi, hg = c // 4, c % 4
        in_maps.append(_prep_core_inputs(inputs, bi, hg, lam, beta_zero))

    import os
    trace = bool(int(os.environ.get("BASS_KERNEL_TRACE", "0")))
    res = run_bass_kernel_spmd(nc, in_maps, list(range(NC)), trace=trace)
    _cached["exec_time_ns"] = res.exec_time_ns
    _cached["trace"] = res.instructions_and_trace
    out = np.zeros((B, N, DIM), np.float32)
    for c in range(NC):
        bi = c // 4
        out[bi] += res.results[c]["yT"].T
    out += bout
    return out


# revision 44
# speedup vs baseline: 1.0114x; 1.0064x over previous
"""DiffAttn2d TRN2 Bass kernel (v2).

Sharding: 8 cores = 2 (batch) x 4 (head-groups of 2 heads / 4 doubled-heads).

Per core (channel-major scores [key j, query i], n=2048, 4 doubled heads):
  - dots^T via row-packed K=16 fp32r matmuls (4 dheads in 4 PE row groups)
  - one ACT exp pass per (jc, dh) dots tile PSUM->SBUF bf16 (the only O(n^2)
    elementwise op; ACT is the bottleneck engine at ~133us busy)
  - attn @ v with the EXP TILE STATIONARY: out z^T[i, c] = sum_j exp[j,i] v[j,c]
    -> 32-row matmuls (cost model charges moving/out rows only).  32 slot
    accumulators interleave in 2 PSUM banks; a single bank-starting matmul
    (jc==0, slot 0/16) lazily zeroes the bank, remaining slots' first writes
    init via the pending-zero flag.  Softmax denominators from 1-row
    ones-matmuls accumulated in [128, 32] PSUM the same way.
  - epilogue in [i-part, c-free] domain on DVE (two pipelined halves):
    reciprocal, lambda-fold, stride-0 broadcast mults, strided even-odd
    subtract, free-dim reduces for LN stats; rsqrt via tiny [128, 8] Ln/Exp
    on ACT (zero table switches)
  - PE f32 transposes (identity moving) bring u' back to [c, i] for
    gamma/beta, sigmoid gating, and the K=32 output projection
  - gates: sigmoid = 1/(1+e^-g): ACT only does e^-g; +1 on Pool; reciprocal
    on DVE for ip0 and ones-divide on Pool for ip1 (tail, off both
    bottleneck engines)
  - y^T partials summed on host (+ bout)
"""
import sys
sys.path.insert(0, "/opt/trn_rl_repo")

import math
import numpy as np
import ml_dtypes

import concourse.bass as bass
import concourse.bacc as bacc_mod
import concourse.mybir as mybir
from concourse.tile import TileContext
from concourse.bass_utils import run_bass_kernel_spmd

F = mybir.dt.float32
R = mybir.dt.float32r
BF = mybir.dt.bfloat16
AF = mybir.ActivationFunctionType
AL = mybir.AluOpType
AX = mybir.AxisListType

H, DH = 8, 16
DEPTH = 1
LAMBDA_INIT = 0.8 - 0.6 * math.exp(-0.3 * DEPTH)
LN_EPS = 1e-5
B, N, DIM = 2, 2048, 256
NC = 8

_cached = {}


def _bcast(ap, n):
    """Append a stride-0 free dim of size n (broadcast along free)."""
    return bass.AP(ap.tensor, ap.offset, list(ap.ap) + [[0, n]])


def build_kernel(beta_zero=True):
    nc = bacc_mod.Bacc()
    xT = nc.declare_dram_parameter("xT", [DIM, N], R, isOutput=False)
    wqp = nc.declare_dram_parameter("wqp", [DIM, 128], R, isOutput=False)
    wkp = nc.declare_dram_parameter("wkp", [DIM, 128], R, isOutput=False)
    wv = nc.declare_dram_parameter("wv", [DIM, 64], R, isOutput=False)
    wgc = nc.declare_dram_parameter("wgc", [DIM, 64], R, isOutput=False)
    wout = nc.declare_dram_parameter("wout", [32, 2, 2, 128], R, isOutput=False)
    ident = nc.declare_dram_parameter("ident", [128, 128], F, isOutput=False)
    lmask = nc.declare_dram_parameter("lmask", [128, 32], F, isOutput=False)
    gam = nc.declare_dram_parameter("gam", [32, 1], F, isOutput=False)
    bet = nc.declare_dram_parameter("bet", [32, 1], F, isOutput=False)
    nbg = nc.declare_dram_parameter("nbg", [32, 2], F, isOutput=False)
    pbg = nc.declare_dram_parameter("pbg", [32, 2], F, isOutput=False)
    epsc = nc.declare_dram_parameter("epsc", [128, 1], F, isOutput=False)
    yT = nc.declare_dram_parameter("yT", [DIM, N], F, isOutput=True)

    with TileContext(nc) as tc:
        with tc.tile_pool(name="pers", bufs=1) as pers, \
             tc.tile_pool(name="ebp", bufs=12) as ebp, \
             tc.tile_pool(name="ep", bufs=2) as ep, \
             tc.tile_pool(name="dp", bufs=2, space="PSUM") as dp, \
             tc.tile_pool(name="zp", bufs=1, space="PSUM") as zp, \
             tc.tile_pool(name="tp", bufs=1, space="PSUM") as tpp, \
             tc.tile_pool(name="sp", bufs=1, space="PSUM") as spp:

            # prewarm ACT exp/ln table set
            warm = pers.tile([1, 8], F, tag="warm")
            nc.vector.memset(warm[:], 0.0)
            nc.scalar.activation(warm[:], warm[:], AF.Exp)

            # ---------------- DMAs (ordered for earliest projections) -------
            xt = pers.tile([128, 2, N], R, tag="xt")
            xTr = xT.rearrange("(f p) n -> p f n", p=128)
            twkp = pers.tile([128, 2, 128], R, tag="twkp")
            nc.sync.dma_start(out=twkp[:], in_=wkp.rearrange("(f p) m -> p f m", p=128))
            twqp = pers.tile([128, 2, 128], R, tag="twqp")
            nc.sync.dma_start(out=twqp[:], in_=wqp.rearrange("(f p) m -> p f m", p=128))
            for c in range(2):
                for f in range(2):
                    nc.sync.dma_start(out=xt[:, f, c * 512:(c + 1) * 512],
                                      in_=xTr[:, f, c * 512:(c + 1) * 512])
            twv = pers.tile([128, 2, 64], R, tag="twv")
            nc.sync.dma_start(out=twv[:], in_=wv.rearrange("(f p) m -> p f m", p=128))
            twg = pers.tile([128, 2, 64], R, tag="twg")
            nc.sync.dma_start(out=twg[:], in_=wgc.rearrange("(f p) m -> p f m", p=128))
            for c in range(2, 4):
                for f in range(2):
                    nc.sync.dma_start(out=xt[:, f, c * 512:(c + 1) * 512],
                                      in_=xTr[:, f, c * 512:(c + 1) * 512])
            tout = pers.tile([32, 2, 2, 128], R, tag="tout")
            nc.sync.dma_start(out=tout[:], in_=wout[:])
            tid = pers.tile([128, 128], F, tag="tid")
            nc.sync.dma_start(out=tid[:], in_=ident[:])
            tlm = pers.tile([128, 32], F, tag="tlm")
            nc.sync.dma_start(out=tlm[:], in_=lmask[:])
            tgam = pers.tile([32, 1], F, tag="tgam")
            nc.sync.dma_start(out=tgam[:], in_=gam[:])
            tbet = pers.tile([32, 1], F, tag="tbet")
            nc.sync.dma_start(out=tbet[:], in_=bet[:])
            tnbg = pers.tile([32, 2], F, tag="tnbg")
            nc.sync.dma_start(out=tnbg[:], in_=nbg[:])
            tpbg = pers.tile([32, 2], F, tag="tpbg")
            nc.sync.dma_start(out=tpbg[:], in_=pbg[:])
            teps = pers.tile([128, 1], F, tag="teps")
            nc.sync.dma_start(out=teps[:], in_=epsc[:])
            tones = pers.tile([128, 1], BF, tag="tones")
            nc.vector.memset(tones[:], 1.0)

            # ---------------- projections ----------------
            # q^T / k^T packed: partition 32d+j (j<16) = channel j of dhead d
            qTp = pers.tile([128, N], R, tag="qTp")
            kTp = pers.tile([128, N], R, tag="kTp")
            vpp = pers.tile([128, 16, 64], BF, tag="vpp")
            sge = pers.tile([32, 4, 1024], F, tag="sge")
            sgq = pers.tile([32, 4, 1024], F, tag="sgq")

            def proj_qk(dst, w, it):
                ps = dp.tile([128, 1024], F, tag="dots")
                for f in range(2):
                    nc.tensor.matmul(ps[:, 0:512], w[:, f, :],
                                     xt[:, f, it * 512:(it + 1) * 512],
                                     start=(f == 0), stop=(f == 1))
                nc.vector.tensor_copy(dst[:, it * 512:(it + 1) * 512], ps[:, 0:512])

            def proj_v(jc):
                ps = dp.tile([128, 1024], F, tag="dots")
                for f in range(2):
                    nc.tensor.matmul(ps[:, 0:64], xt[:, f, jc * 128:(jc + 1) * 128],
                                     twv[:, f, :], start=(f == 0), stop=(f == 1))
                nc.vector.tensor_copy(vpp[:, jc, :], ps[:, 0:64])

            def gates_mm(ip, b):
                ps = dp.tile([128, 1024], F, tag="dots")
                for nt in range(2):
                    for f in range(2):
                        nc.tensor.matmul(
                            ps[0:32, nt * 512:(nt + 1) * 512],
                            twg[:, f, 32 * b:32 * b + 32],
                            xt[:, f, ip * 1024 + nt * 512:ip * 1024 + (nt + 1) * 512],
                            start=(f == 0), stop=(f == 1))
                nc.scalar.activation(sge[:, 2 * ip + b, :], ps[0:32, :], AF.Exp,
                                     scale=-1.0, bias=tnbg[:, b:b + 1])

            proj_qk(kTp, twkp, 0)
            proj_qk(qTp, twqp, 0)
            proj_qk(qTp, twqp, 1)
            proj_v(0)

            def proj_rest():
                items = [lambda: proj_v(1),
                         lambda: proj_v(2), lambda: proj_v(3),
                         lambda: proj_qk(kTp, twkp, 1), lambda: proj_v(4), lambda: proj_v(5),
                         lambda: proj_qk(kTp, twkp, 2), lambda: proj_v(6), lambda: proj_v(7),
                         lambda: proj_qk(kTp, twkp, 3), lambda: proj_v(8), lambda: proj_v(9),
                         lambda: proj_qk(qTp, twqp, 2), lambda: proj_v(10), lambda: proj_v(11),
                         lambda: proj_qk(qTp, twqp, 3), lambda: proj_v(12), lambda: proj_v(13),
                         lambda: proj_v(14), lambda: proj_v(15),
                         lambda: gates_mm(0, 0), lambda: gates_mm(0, 1)]
                for i, it in enumerate(items):
                    it()
                    if i >= 10 or i % 2 == 1:
                        yield

            # ---------------- attention ----------------
            saved = {}

            def attention(ip, interleave=None):
                i0 = ip * 1024
                zreg = zp.tile([128, 32, 32], F, tag="zreg")
                sreg = spp.tile([128, 32], F, tag="sreg")

                def zs_mm(jc, ebs):
                    # z/s matmuls lag the dots of the NEXT jc so the PE never
                    # blocks on the current jc's last exp
                    for dh in range(4):
                        pair = dh // 2
                        eb = ebs[dh]
                        for ib in range(8):
                            slot = ib * 4 + dh
                            st = eb[:, ib * 128:(ib + 1) * 128]
                            nc.tensor.matmul(
                                zreg[:, slot, :], st,
                                vpp[:, jc, 32 * pair:32 * pair + 32],
                                start=(jc == 0 and slot in (0, 16)),
                                stop=(jc == 15),
                                skip_group_check=True,
                            )
                            nc.tensor.matmul(
                                sreg[:, slot:slot + 1], st, tones[:],
                                start=(jc == 0 and slot == 0),
                                stop=(jc == 15),
                                skip_group_check=True,
                            )

                prev = None
                for jc in range(16):
                    if interleave is not None:
                        next(interleave, None)
                    j0 = jc * 128
                    ebs = []
                    for dh in range(4):
                        r0 = 32 * dh
                        dt_ = dp.tile([128, 1024], F, tag="dots")
                        for h in range(2):
                            nc.tensor.matmul(
                                dt_[:, h * 512:(h + 1) * 512],
                                kTp[r0:r0 + 16, j0:j0 + 128],
                                qTp[r0:r0 + 16, i0 + h * 512:i0 + (h + 1) * 512],
                                start=True, stop=True,
                                tile_position=(r0, 0),
                            )
                        eb = ebp.tile([128, 1024], BF, tag="ebf")
                        nc.scalar.activation(eb[:], dt_[:], AF.Exp)
                        ebs.append(eb)
                    if prev is not None:
                        zs_mm(jc - 1, prev)
                    prev = ebs
                zs_mm(15, prev)
                saved[ip] = (zreg, sreg)

            def epi_dve(ip):
                """u/LN chain in [i, c] domain, split in two pipelined halves
                (pairslots 0:8 | 8:16 = slots 0:16 | 16:32)."""
                zreg, sreg = saved[ip]
                ucs = []
                for hb in range(2):
                    s0, ps0 = 16 * hb, 8 * hb
                    sfx = f"{ip}{hb}"
                    rinv = ep.tile([128, 16], F, tag=f"rinv{hb}", name=f"rinv{sfx}")
                    nc.vector.reciprocal(rinv[:], sreg[:, s0:s0 + 16])
                    rinl = ep.tile([128, 16], F, tag=f"rinl{hb}", name=f"rinl{sfx}")
                    nc.vector.tensor_tensor(rinl[:], rinv[:], tlm[:, s0:s0 + 16], AL.mult)
                    zr = ep.tile([128, 16, 32], F, tag=f"zr{hb}", name=f"zr{sfx}")
                    nc.vector.tensor_tensor(zr[:], zreg[:, s0:s0 + 16, :],
                                            _bcast(rinl[:], 32), AL.mult)
                    u = ep.tile([128, 8, 32], F, tag=f"u{hb}", name=f"u{sfx}")
                    nc.vector.tensor_tensor(u[:], zr[:, 0:16:2, :], zr[:, 1:16:2, :],
                                            AL.subtract)
                    usq = ep.tile([128, 8, 32], F, tag=f"usq{hb}", name=f"usq{sfx}")
                    nc.vector.tensor_tensor(usq[:], u[:], u[:], AL.mult)
                    s1 = ep.tile([128, 8], F, tag=f"s1{hb}", name=f"s1{sfx}")
                    nc.vector.tensor_reduce(s1[:], u[:], AX.X, AL.add)
                    s2 = ep.tile([128, 8], F, tag=f"s2{hb}", name=f"s2{sfx}")
                    nc.vector.tensor_reduce(s2[:], usq[:], AX.X, AL.add)
                    mu = ep.tile([128, 8], F, tag=f"mu{hb}", name=f"mu{sfx}")
                    nc.vector.tensor_scalar(mu[:], s1[:], 1.0 / 32.0, None, AL.mult)
                    var = ep.tile([128, 8], F, tag=f"var{hb}", name=f"var{sfx}")
                    nc.vector.tensor_tensor(var[:], mu[:], mu[:], AL.mult)
                    nc.vector.tensor_scalar(var[:], var[:], 32.0, None, AL.mult)
                    nc.vector.tensor_tensor(var[:], s2[:], var[:], AL.subtract)
                    w = ep.tile([128, 8], F, tag=f"w{hb}", name=f"w{sfx}")
                    nc.scalar.activation(var[:], var[:], AF.Ln, scale=1.0 / 32.0,
                                         bias=teps[:])
                    nc.scalar.activation(w[:], var[:], AF.Exp, scale=-0.5)
                    uc = ep.tile([128, 8, 32], F, tag=f"uc{hb}", name=f"uc{sfx}")
                    nc.vector.tensor_tensor(uc[:], u[:], _bcast(mu[:], 32), AL.subtract)
                    nc.vector.tensor_tensor(uc[:], uc[:], _bcast(w[:], 32), AL.mult)
                    ucs.append(uc)
                saved[ip] = ucs

            def epi_out(ip):
                """u'^T via PE transposes, gamma/beta + gating on DVE, output
                projection per 512-col chunk; yields between chunks."""
                ucs = saved[ip]
                gr2 = ep.tile([32, 2, 1024], R, tag="gr2", name=f"gr2_{ip}")
                for nt in range(2):
                    for ihq in range(2 * nt, 2 * nt + 2):
                        tpt = tpp.tile([32, 2, 256], F, tag="tpt")
                        for pair in range(2):
                            for ib2 in range(2):
                                ib = ihq * 2 + ib2
                                psl = (ib * 2 + pair) % 8
                                nc.tensor.transpose(
                                    tpt[:, pair, ib2 * 128:(ib2 + 1) * 128],
                                    ucs[ihq // 2][:, psl, :], tid[:])
                        c0 = ihq * 256
                        if beta_zero:
                            # gamma folded into wout on the host; beta==0:
                            # gating is a single mult straight off the psum
                            nc.vector.tensor_tensor(
                                gr2[:, :, c0:c0 + 256], tpt[:],
                                sgq[:, 2 * ip:2 * ip + 2, c0:c0 + 256], AL.mult)
                        else:
                            gg = ep.tile([32, 2, 256], F, tag="gg",
                                         name=f"gg_{ip}{ihq}")
                            nc.vector.tensor_scalar(gg[:], tpt[:], tgam[:],
                                                    tbet[:], AL.mult, AL.add)
                            nc.vector.tensor_tensor(
                                gr2[:, :, c0:c0 + 256], gg[:],
                                sgq[:, 2 * ip:2 * ip + 2, c0:c0 + 256], AL.mult)
                        yield
                    yp = dp.tile([128, 1024], F, tag="dots")
                    for oh in range(2):
                        for pair in range(2):
                            nc.tensor.matmul(yp[:, oh * 512:(oh + 1) * 512],
                                             tout[:, pair, oh, :],
                                             gr2[:, pair, nt * 512:(nt + 1) * 512],
                                             start=(pair == 0), stop=(pair == 1))
                    ys = ep.tile([128, 1024], F, tag=f"ys{nt}", name=f"ys{nt}_{ip}")
                    if ip == 1 and nt == 0:
                        # keep DVE free for the ongoing epilogue cascade:
                        # copies via idle ACT
                        nc.scalar.copy(ys[:, 0:512], yp[:, 0:512])
                        nc.scalar.copy(ys[:, 512:1024], yp[:, 512:1024])
                    elif ip == 1:
                        # last chunk: split across ACT and DVE
                        nc.scalar.copy(ys[:, 0:512], yp[:, 0:512])
                        nc.vector.tensor_copy(ys[:, 512:1024], yp[:, 512:1024])
                    else:
                        nc.vector.tensor_copy(ys[:], yp[:])
                    c0 = ip * 1024 + nt * 512
                    nc.sync.dma_start(out=yT[0:128, c0:c0 + 512], in_=ys[:, 0:512])
                    eng = nc.scalar if ip == 1 else nc.sync
                    eng.dma_start(out=yT[128:256, c0:c0 + 512], in_=ys[:, 512:1024])
                    yield

            gen_proj = proj_rest()
            attention(0, interleave=gen_proj)
            for _ in gen_proj:
                pass
            # finish sigmoid for ip0 off the critical engines
            nc.gpsimd.tensor_scalar(sge[:, 0:2, :], sge[:, 0:2, :], 1.0, None, AL.add)
            nc.vector.reciprocal(sgq[:, 0:2, :], sge[:, 0:2, :])
            epi_dve(0)
            gen0 = epi_out(0)

            class InterleaveAt:
                """Step gen0 only during the later jc's of attention(1)."""
                def __init__(self, gen, start):
                    self.gen, self.start, self.jc = gen, start, 0
                def __next__(self):
                    if self.jc >= self.start:
                        next(self.gen, None)
                    self.jc += 1
                    return None

            attention(1, interleave=InterleaveAt(gen0, 8))
            for _ in gen0:
                pass
            # gates for ip1: the whole sigmoid runs on ACT inside the
            # post-attention ACT gap: sig = exp(-ln(exp(-(g+bg)) + 1))
            gates_mm(1, 0)
            gates_mm(1, 1)
            nc.scalar.activation(sge[:, 2:4, :], sge[:, 2:4, :], AF.Ln, bias=1.0)
            nc.scalar.activation(sgq[:, 2:4, :], sge[:, 2:4, :], AF.Exp, scale=-1.0)
            epi_dve(1)
            for _ in epi_out(1):
                pass

    nc.finalize()
    return nc


def _prep_core_inputs(inputs, bi, hg, lam, beta_zero=True):
    scale = DH ** -0.5
    x = np.asarray(inputs["x"], np.float32)
    Wq = np.asarray(inputs["Wq"], np.float32)
    Wkv = np.asarray(inputs["Wkv"], np.float32)
    Wout = np.asarray(inputs["Wout"], np.float32)
    Wg = np.asarray(inputs["Wg"], np.float32)
    bg = np.asarray(inputs["bg"], np.float32)
    g_ = np.asarray(inputs["ln_gamma"], np.float32)
    b_ = np.asarray(inputs["ln_beta"], np.float32)
    li = np.float32(1.0 - LAMBDA_INIT)

    c0 = 64 * hg
    wq_c = Wq[:, c0:c0 + 64] * scale
    wk_c = Wkv[:, c0:c0 + 64]
    wv_c = Wkv[:, 256 + c0:256 + c0 + 64]
    wg_c = Wg[:, c0:c0 + 64]
    wout_c = Wout[c0:c0 + 64, :]

    wqp = np.zeros((256, 128), np.float32)
    wkp = np.zeros((256, 128), np.float32)
    for d in range(4):
        wqp[:, 32 * d:32 * d + 16] = wq_c[:, 16 * d:16 * d + 16]
        wkp[:, 32 * d:32 * d + 16] = wk_c[:, 16 * d:16 * d + 16]

    gvec = (g_[0:32] * li).astype(np.float32)
    woutp = np.zeros((32, 2, 2, 128), np.float32)
    for pair in range(2):
        for oh in range(2):
            woutp[:, pair, oh, :] = wout_c[32 * pair:32 * pair + 32,
                                           128 * oh:128 * oh + 128]
            if beta_zero:
                # gamma (and the 1-lambda_init factor) folded into wout
                woutp[:, pair, oh, :] *= gvec[:, None]

    lmask = np.ones((128, 32), np.float32)
    for slot in range(32):
        if slot % 4 in (1, 3):
            lmask[:, slot] = lam

    nbg = np.zeros((32, 2), np.float32)
    pbg = np.zeros((32, 2), np.float32)
    for b in range(2):
        nbg[:, b] = -bg[c0 + 32 * b:c0 + 32 * b + 32]
        pbg[:, b] = bg[c0 + 32 * b:c0 + 32 * b + 32]

    return {
        "xT": np.ascontiguousarray(x[bi].T),
        "wqp": wqp, "wkp": wkp,
        "wv": np.ascontiguousarray(wv_c),
        "wgc": np.ascontiguousarray(wg_c),
        "wout": woutp,
        "ident": np.eye(128, dtype=np.float32),
        "lmask": lmask,
        "gam": (g_[0:32] * li).reshape(32, 1).astype(np.float32),
        "bet": (b_[0:32] * li).reshape(32, 1).astype(np.float32),
        "nbg": nbg, "pbg": pbg,
        "epsc": np.full((128, 1), LN_EPS, np.float32),
    }


def kernel(**inputs) -> np.ndarray:
    lq1 = np.asarray(inputs["lq1"], np.float64)
    lk1 = np.asarray(inputs["lk1"], np.float64)
    lq2 = np.asarray(inputs["lq2"], np.float64)
    lk2 = np.asarray(inputs["lk2"], np.float64)
    lam = float(np.exp(np.sum(lq1 * lk1)) - np.exp(np.sum(lq2 * lk2)) + LAMBDA_INIT)
    bout = np.asarray(inputs["bout"], np.float32)
    beta_zero = bool(np.all(np.asarray(inputs["ln_beta"]) == 0.0))

    key = ("nc", beta_zero)
    if key not in _cached:
        _cached[key] = build_kernel(beta_zero)
    nc = _cached[key]

    in_maps = []
    for c in range(NC):
        bi, hg = c // 4, c % 4
        in_maps.append(_prep_core_inputs(inputs, bi, hg, lam, beta_zero))

    import os
    trace = bool(int(os.environ.get("BASS_KERNEL_TRACE", "0")))
    res = run_bass_kernel_spmd(nc, in_maps, list(range(NC)), trace=trace)
    _cached["exec_time_ns"] = res.exec_time_ns
    _cached["trace"] = res.instructions_and_trace
    out = np.zeros((B, N, DIM), np.float32)
    for c in range(NC):
        bi = c // 4
        out[bi] += res.results[c]["yT"].T
    out += bout
    return out
